# revision 28
# baseline (speedup 1.0000x reference)
"""Trainium2 Bass kernel: multi-head attention (B,C,S,H)=(2,4,1024,2048), NH=16, HD=128.

Strategy: pure data-parallel over the 8 B*C batch elements -> 8 NeuronCores,
no collectives.  Per core (v2, software-pipelined phases):
  phase A:  Q^T,K^T projection (transposed [head-dim, seq] layout, lhsT = w
            tiles, rhs = x^T tiles) with RoPE fused into the PSUM eviction.
            Startup DMAs ordered so the first matmul's inputs land first.
  phase BV0: attention for q-chunk 0 of every head, with the V projection
            matmuls interleaved as PE filler so the ScalarE exp stream hides
            behind matmuls instead of stalling the PE.
  phase BV1: attention for q-chunk 1, with the O-projection matmuls for
            q-chunk 0 interleaved as filler (one output d-tile per head).
  epilogue: O-projection for q-chunk 1, outputs DMA'd per-tile in bf16
            (host upcasts to f32).
All matmuls bf16 (fp8 fails the 2e-2 gate: measured 3-6% rel err), f32
accumulation in PSUM.  Host side pre-transposes/pre-tiles/casts inputs so
every DMA is partition-major contiguous.
"""

import numpy as np
import ml_dtypes

try:
    import concourse  # noqa: F401
except ImportError:
    import sys
    sys.path.insert(0, "/opt/trn_rl_repo")

BF = ml_dtypes.bfloat16

B, C, S, H = 2, 4, 1024, 2048
NH, HD, NENC = 16, 128, 1008
NCORES = 8
KT = H // 128          # 16 contraction tiles for the projections
DT = H // 128          # 16 output d-tiles (heads) for Q/K
ST = S // 128          # 8 seq tiles
SCHUNK = 512
NSC = S // SCHUNK      # 2 seq chunks
SCALE = 1.0 / float(np.sqrt(HD))
AV_LAG = 2             # kt8 distance between scores and AV consumption
PSS = 3                # score psum ring


class Filler:
    """Wraps a generator that emits one filler matmul per next()."""

    def __init__(self, gen):
        self.gen = gen
        self.done = False

    def take(self, n):
        for _ in range(n):
            if self.done:
                return
            try:
                next(self.gen)
            except StopIteration:
                self.done = True
                return

    def drain(self):
        while not self.done:
            self.take(1)


def build_nc():
    import concourse.bass as bass
    import concourse.mybir as mybir
    import concourse.tile as tile
    from concourse import bacc

    from concourse import bass_isa

    f32 = mybir.dt.float32
    bf16 = mybir.dt.bfloat16

    nc = bacc.Bacc(None, target_bir_lowering=False, debug=False)

    xT = nc.dram_tensor("xT", [128, KT * S], bf16, kind="ExternalInput")
    wq = nc.dram_tensor("wq", [128, DT * H], bf16, kind="ExternalInput")
    wk = nc.dram_tensor("wk", [128, DT * H], bf16, kind="ExternalInput")
    wv = nc.dram_tensor("wv", [128, 4 * KT * 512], bf16, kind="ExternalInput")
    wo = nc.dram_tensor("wo", [128, DT * H], bf16, kind="ExternalInput")
    cosT = nc.dram_tensor("cosT", [128, S], f32, kind="ExternalInput")
    sinTs = nc.dram_tensor("sinTs", [128, S], f32, kind="ExternalInput")
    out = nc.dram_tensor("out", [H, S], bf16, kind="ExternalOutput")

    with tile.TileContext(nc) as tc:
        import contextlib
        with contextlib.ExitStack() as ctx:
            # ---- persistent SBUF tiles -------------------------------------
            persist = ctx.enter_context(tc.tile_pool(name="persist", bufs=1))
            qT_sb = persist.tile([128, NH * S], bf16, tag="qT")
            kT_sb = persist.tile([128, NH * S], bf16, tag="kT")
            v_sb = persist.tile([128, ST * H], bf16, tag="v")
            attnT_sb = persist.tile([128, NH * S], bf16, tag="attnT")

            # xT lives through phase A and BV0 (V projection reads it)
            x_pool = ctx.enter_context(tc.tile_pool(name="xpool", bufs=1))
            xT_sb = x_pool.tile([128, KT * S], bf16, tag="xT")

            # wv stream lives A..BV0: ring of half-group chunks (8KB each)
            wv_pool = ctx.enter_context(tc.tile_pool(name="wv_stream", bufs=3))

            def load_wv_chunk(g, half):
                t = wv_pool.tile([128, 8 * 512], bf16, tag="wv",
                                 name=f"wv_{g}_{half}")
                nc.sync.dma_start(
                    out=t[:],
                    in_=wv[:, g * KT * 512 + half * 8 * 512:
                           g * KT * 512 + (half + 1) * 8 * 512])
                return t

            # ---- phase A: Q^T / K^T projection with fused RoPE -------------
            with tc.tile_pool(name="trig", bufs=1) as trig_pool, \
                 tc.tile_pool(name="wqk_stream", bufs=2) as wqk_pool, \
                 tc.tile_pool(name="rope_scratch", bufs=1) as rope_pool, \
                 tc.tile_pool(name="psumA", bufs=6, space="PSUM") as psA:

                cos_sb = trig_pool.tile([128, S], f32, tag="cos")
                sin_sb = trig_pool.tile([128, S], f32, tag="sin")

                # startup-critical DMA order: the first kt-halves of the
                # first two q weight tiles and the first x k-tile land
                # first; cos/sin (needed only at first eviction) go last.
                wt_pre = {}
                wt0 = wqk_pool.tile([128, KT * 128], bf16, tag="wqk",
                                    name="wt_pre_0")
                wt1 = wqk_pool.tile([128, KT * 128], bf16, tag="wqk",
                                    name="wt_pre_1")
                nc.sync.dma_start(out=wt0[:, 0:512], in_=wq[:, 0:512])
                nc.sync.dma_start(out=xT_sb[:, 0:S], in_=xT[:, 0:S])
                nc.sync.dma_start(out=wt1[:, 0:512], in_=wq[:, H:H + 512])
                nc.sync.dma_start(out=xT_sb[:, S:2 * S], in_=xT[:, S:2 * S])
                nc.sync.dma_start(out=wt0[:, 512:1024], in_=wq[:, 512:1024])
                nc.sync.dma_start(out=wt1[:, 512:1024],
                                  in_=wq[:, H + 512:H + 1024])
                nc.sync.dma_start(out=wt0[:, 1024:2048], in_=wq[:, 1024:2048])
                nc.sync.dma_start(out=wt1[:, 1024:2048],
                                  in_=wq[:, H + 1024:H + 2048])
                for kt in range(2, KT):
                    nc.sync.dma_start(out=xT_sb[:, kt * S:(kt + 1) * S],
                                      in_=xT[:, kt * S:(kt + 1) * S])
                nc.sync.dma_start(out=cos_sb[:], in_=cosT[:])
                nc.sync.dma_start(out=sin_sb[:], in_=sinTs[:])
                wt_pre[0] = wt0
                wt_pre[1] = wt1
                # prefetch V group 0 so the BV0 prologue starts immediately
                wv_g0 = [load_wv_chunk(0, 0), load_wv_chunk(0, 1)]

                def rope_evict(psum, dst_ap, sc):
                    # dst = psum*cos + shifted(psum)*sin_signed over this chunk
                    cs = cos_sb[:, sc * SCHUNK:(sc + 1) * SCHUNK]
                    ss = sin_sb[:, sc * SCHUNK:(sc + 1) * SCHUNK]
                    m1 = rope_pool.tile([128, SCHUNK], f32, tag="rope_m1")
                    nc.vector.tensor_mul(m1[:], psum[:], cs)
                    m2 = rope_pool.tile([128, SCHUNK], f32, tag="rope_tmp")
                    nc.vector.tensor_mul(m2[0:64, :], psum[64:128, :],
                                         ss[0:64, :])
                    nc.vector.tensor_mul(m2[64:128, :], psum[0:64, :],
                                         ss[64:128, :])
                    nc.vector.tensor_add(dst_ap, m1[:], m2[:])

                # kt-outer warmup over the first two q d-tiles: 4 psum groups
                # accumulate in parallel so each matmul is gated only on its
                # own xT k-tile DMA, not on the whole xT stream.
                warm = []
                for dt in (0, 1):
                    for sc in range(NSC):
                        ps = psA.tile([128, SCHUNK], f32, tag="psA",
                                      name=f"psA_warm_{dt}_{sc}")
                        warm.append((dt, sc, ps))
                for kt in range(KT):
                    for dt, sc, ps in warm:
                        nc.tensor.matmul(
                            ps[:],
                            wt_pre[dt][:, kt * 128:(kt + 1) * 128],
                            xT_sb[:, kt * S + sc * SCHUNK:
                                  kt * S + (sc + 1) * SCHUNK],
                            start=(kt == 0), stop=(kt == KT - 1),
                        )
                for dt, sc, ps in warm:
                    rope_evict(ps, qT_sb[:, dt * S + sc * SCHUNK:
                                         dt * S + (sc + 1) * SCHUNK], sc)

                for which, wdram, dst_sb in (("q", wq, qT_sb), ("k", wk, kT_sb)):
                    for dt in range(DT):
                        if which == "q" and dt in wt_pre:
                            continue  # handled by the kt-outer warmup
                        else:
                            wt = wqk_pool.tile([128, KT * 128], bf16, tag="wqk",
                                               name=f"wt_{which}_{dt}")
                            for c in range(2):
                                nc.sync.dma_start(
                                    out=wt[:, c * 1024:(c + 1) * 1024],
                                    in_=wdram[:, dt * H + c * 1024:
                                              dt * H + (c + 1) * 1024])
                        for sc in range(NSC):
                            ps = psA.tile([128, SCHUNK], f32, tag="psA")
                            for kt in range(KT):
                                nc.tensor.matmul(
                                    ps[:],
                                    wt[:, kt * 128:(kt + 1) * 128],
                                    xT_sb[:, kt * S + sc * SCHUNK:
                                          kt * S + (sc + 1) * SCHUNK],
                                    start=(kt == 0), stop=(kt == KT - 1),
                                )
                            dst = dst_sb[:, dt * S + sc * SCHUNK:
                                         dt * S + (sc + 1) * SCHUNK]
                            rope_evict(ps, dst, sc)

            # ---- attention pools (BV0 + BV1) --------------------------------
            wo_pool = ctx.enter_context(tc.tile_pool(name="wo_stream", bufs=3))

            def load_wo(qc, ot):
                t = wo_pool.tile([128, KT * 128], bf16, tag="wo",
                                 name=f"wo_{qc}_{ot}")
                nc.sync.dma_start(out=t[:], in_=wo[:, ot * H:(ot + 1) * H])
                return t

            with tc.tile_pool(name="expS", bufs=4) as expS_pool, \
                 tc.tile_pool(name="esum", bufs=2) as esum_pool, \
                 tc.tile_pool(name="den", bufs=1) as den_pool, \
                 tc.tile_pool(name="norm", bufs=1) as norm_pool, \
                 tc.tile_pool(name="psS", bufs=PSS, space="PSUM") as psS, \
                 tc.tile_pool(name="psAV", bufs=2, space="PSUM") as psAV, \
                 tc.tile_pool(name="psFill", bufs=3, space="PSUM") as psFill:

                def attention(h, qc, filler, f1, f2, ftail, fden):
                    ps_av = psAV.tile([128, SCHUNK], f32, tag="psAV",
                                      name=f"psav_{h}_{qc}")
                    esum = esum_pool.tile([128, SCHUNK], bf16, tag="esum",
                                          name=f"esum_{h}_{qc}")
                    exp_tiles = {}

                    def av(j):
                        nc.tensor.matmul(
                            ps_av[:],
                            v_sb[:, j * H + h * 128: j * H + (h + 1) * 128],
                            exp_tiles[j][:],
                            start=(j == 0), stop=(j == ST - 1),
                        )

                    for kt8 in range(ST):
                        ps_s = psS.tile([128, SCHUNK], f32, tag="psS",
                                        name=f"pss_{h}_{qc}_{kt8}")
                        nc.tensor.matmul(
                            ps_s[:],
                            kT_sb[:, h * S + kt8 * 128: h * S + (kt8 + 1) * 128],
                            qT_sb[:, h * S + qc * SCHUNK:
                                  h * S + (qc + 1) * SCHUNK],
                            start=True, stop=True,
                        )
                        e = expS_pool.tile([128, SCHUNK], bf16, tag="expS",
                                           name=f"exp_{h}_{qc}_{kt8}")
                        nc.scalar.activation(
                            e[:], ps_s[:],
                            func=mybir.ActivationFunctionType.Exp,
                            scale=SCALE,
                        )
                        exp_tiles[kt8] = e
                        if kt8 == 0:
                            nc.vector.tensor_copy(esum[:], e[:])
                        else:
                            nc.vector.tensor_add(esum[:], esum[:], e[:])
                        filler.take(f1)
                        if kt8 >= AV_LAG:
                            av(kt8 - AV_LAG)
                        filler.take(f2)
                    for j in range(ST - AV_LAG, ST):
                        av(j)
                        filler.take(ftail)
                    filler.take(fden)
                    # denominator broadcast-sum across partitions on the
                    # otherwise-idle GpSimd engine (frees 32 PE matmuls)
                    den = den_pool.tile([128, SCHUNK], f32, tag="den",
                                        name=f"den_{h}_{qc}")
                    nc.gpsimd.partition_all_reduce(
                        den[:], esum[:], 128, bass_isa.ReduceOp.add)
                    recipb = norm_pool.tile([128, SCHUNK], f32, tag="recipb",
                                            name=f"recipb_{h}_{qc}")
                    nc.vector.reciprocal_approx_fast(out=recipb[:], in_=den[:])
                    nc.vector.tensor_mul(
                        attnT_sb[:, h * S + qc * SCHUNK:
                                 h * S + (qc + 1) * SCHUNK],
                        ps_av[:], recipb[:])

                # ---- BV0: attention qc=0, V projection as filler ---------
                def v_gen():
                    chunks = wv_g0
                    for g in range(4):
                        nxt = None
                        for st in range(ST):
                            if st == 4 and g < 3:
                                nxt = [load_wv_chunk(g + 1, 0),
                                       load_wv_chunk(g + 1, 1)]
                            ps = psFill.tile([128, SCHUNK], f32, tag="psF",
                                             name=f"psv_{g}_{st}")
                            for kt in range(KT):
                                ch = chunks[kt // 8]
                                nc.tensor.matmul(
                                    ps[:],
                                    xT_sb[:, kt * S + st * 128:
                                          kt * S + (st + 1) * 128],
                                    ch[:, (kt % 8) * 512:(kt % 8 + 1) * 512],
                                    start=(kt == 0), stop=(kt == KT - 1),
                                )
                                yield
                            # DVE copy: keeps ScalarE free for the exp stream
                            nc.vector.tensor_copy(
                                v_sb[:, st * H + g * 512: st * H + (g + 1) * 512],
                                ps[:])
                        chunks = nxt

                o_pre = {}
                vf = Filler(v_gen())
                vf.take(128)  # V group 0 prologue (heads 0-3 need it)
                for h in range(NH):
                    if h == NH - 2:
                        # prefetch the first two O-proj weight tiles so BV1's
                        # filler doesn't head-of-line-block the PE
                        o_pre[0] = {0: load_wo(0, 0), 1: load_wo(0, 1)}
                    attention(h, 0, vf, f1=2, f2=1, ftail=2, fden=4)
                vf.drain()

                # ---- BV1: attention qc=1, O projection (qc=0) as filler --
                ostage_cm = tc.tile_pool(name="ostage", bufs=1)
                ostage_pool = ostage_cm.__enter__()

                def o_gen(qc, tiles):
                    for ot in range(DT):
                        if ot not in tiles:
                            tiles[ot] = load_wo(qc, ot)
                        wot = tiles.pop(ot)
                        ps = psFill.tile([128, SCHUNK], f32, tag="psF",
                                         name=f"pso_{qc}_{ot}")
                        for dt in range(DT):
                            nc.tensor.matmul(
                                ps[:],
                                wot[:, dt * 128:(dt + 1) * 128],
                                attnT_sb[:, dt * S + qc * SCHUNK:
                                         dt * S + (qc + 1) * SCHUNK],
                                start=(dt == 0), stop=(dt == DT - 1),
                            )
                            if dt == 7 and ot + 2 < DT:
                                tiles[ot + 2] = load_wo(qc, ot + 2)
                            yield
                        o_bf = ostage_pool.tile([128, SCHUNK], bf16,
                                                tag="ostage",
                                                name=f"ost_{qc}_{ot}")
                        nc.scalar.copy(o_bf[:], ps[:])
                        # post from ScalarE right after its own evict: no
                        # cross-engine semaphore, no GpSimd drain at exit
                        nc.scalar.dma_start(
                            out=out[ot * 128:(ot + 1) * 128,
                                    qc * SCHUNK:(qc + 1) * SCHUNK],
                            in_=o_bf[:])

                of0 = Filler(o_gen(0, o_pre[0]))
                for h in range(NH):
                    if h == NH - 2:
                        o_pre[1] = {0: load_wo(1, 0), 1: load_wo(1, 1)}
                    attention(h, 1, of0, f1=1, f2=1, ftail=0, fden=0)
                of0.drain()

                # ---- epilogue: O projection qc=1 --------------------------
                of1 = Filler(o_gen(1, o_pre[1]))
                of1.drain()

                ostage_cm.__exit__(None, None, None)
    nc.finalize()
    return nc


def _prep_core_inputs(x_bc, wq_t, wk_t, wv_t, wo_t, cosT_p, sinTs_p):
    # x_bc: (S, H) f32 -> xT partition-major [128, KT*S] bf16
    xT_p = np.ascontiguousarray(
        x_bc.T.reshape(KT, 128, S).transpose(1, 0, 2).reshape(128, KT * S)
    ).astype(BF)
    return {
        "xT": xT_p, "wq": wq_t, "wk": wk_t, "wv": wv_t, "wo": wo_t,
        "cosT": cosT_p, "sinTs": sinTs_p,
    }


def _prep_shared(cos, sin, w_qkv, w_o):
    def dtile_major(w):  # (H, 2048) -> [128, DT*H], lhsT tiles (dt, kt)
        return np.ascontiguousarray(
            w.reshape(KT, 128, DT, 128).transpose(1, 2, 0, 3).reshape(128, DT * H)
        ).astype(BF)

    wq_t = dtile_major(w_qkv[:, :H])
    wk_t = dtile_major(w_qkv[:, H:2 * H])
    wo_t = dtile_major(w_o)
    wv_t = np.ascontiguousarray(
        w_qkv[:, 2 * H:].reshape(KT, 128, 4, 512).transpose(1, 2, 0, 3)
        .reshape(128, 4 * KT * 512)
    ).astype(BF)

    cos_p = np.ones((S, HD), np.float32)
    cos_p[:NENC] = cos
    sin_p = np.zeros((S, HD), np.float32)
    sin_p[:NENC] = sin
    cosT_p = np.ascontiguousarray(cos_p.T)
    sinT = sin_p.T.copy()
    sinTs_p = np.concatenate([-sinT[:64], sinT[64:]], axis=0)
    sinTs_p = np.ascontiguousarray(sinTs_p)
    return wq_t, wk_t, wv_t, wo_t, cosT_p, sinTs_p


_CACHED_NC = None


def kernel(hidden_states, cos, sin, w_qkv, w_o):
    global _CACHED_NC
    from concourse.bass_utils import run_bass_kernel_spmd

    hidden_states = np.asarray(hidden_states, dtype=np.float32)
    cos = np.asarray(cos, dtype=np.float32)
    sin = np.asarray(sin, dtype=np.float32)
    w_qkv = np.asarray(w_qkv, dtype=np.float32)
    w_o = np.asarray(w_o, dtype=np.float32)

    shared = _prep_shared(cos, sin, w_qkv, w_o)
    xs = hidden_states.reshape(B * C, S, H)
    in_maps = [_prep_core_inputs(xs[i], *shared) for i in range(NCORES)]

    if _CACHED_NC is None:
        _CACHED_NC = build_nc()
    res = run_bass_kernel_spmd(_CACHED_NC, in_maps, list(range(NCORES)))

    out_full = np.empty((B * C, S, H), np.float32)
    for i in range(NCORES):
        out_full[i] = np.asarray(res.results[i]["out"]).astype(np.float32).T
    return out_full.reshape(B, C, S, H)


# revision 36
# speedup vs baseline: 1.3022x; 1.3022x over previous
"""Trainium2 Bass kernel: multi-head attention (B,C,S,H)=(2,4,1024,2048), NH=16, HD=128.

Strategy: pure data-parallel over the 8 B*C batch elements -> 8 NeuronCores,
no collectives.  Per core (v2, software-pipelined phases):
  phase A:  Q^T,K^T projection (transposed [head-dim, seq] layout, lhsT = w
            tiles, rhs = x^T tiles) with RoPE fused into the PSUM eviction.
            Startup DMAs ordered so the first matmul's inputs land first.
  phase BV0: attention for q-chunk 0 of every head, with the V projection
            matmuls interleaved as PE filler so the ScalarE exp stream hides
            behind matmuls instead of stalling the PE.
  phase BV1: attention for q-chunk 1, with the O-projection matmuls for
            q-chunk 0 interleaved as filler (one output d-tile per head).
  epilogue: O-projection for q-chunk 1, outputs DMA'd per-tile in bf16
            (host upcasts to f32).
All matmuls bf16 (fp8 fails the 2e-2 gate: measured 3-6% rel err), f32
accumulation in PSUM.  Host side pre-transposes/pre-tiles/casts inputs so
every DMA is partition-major contiguous.
"""

import numpy as np
import ml_dtypes

try:
    import concourse  # noqa: F401
except ImportError:
    import sys
    sys.path.insert(0, "/opt/trn_rl_repo")

BF = ml_dtypes.bfloat16

B, C, S, H = 2, 4, 1024, 2048
NH, HD, NENC = 16, 128, 1008
NCORES = 8
KT = H // 128          # 16 contraction tiles for the projections
DT = H // 128          # 16 output d-tiles (heads) for Q/K
ST = S // 128          # 8 seq tiles
SCHUNK = 512
NSC = S // SCHUNK      # 2 seq chunks
SCALE = 1.0 / float(np.sqrt(HD))
AV_LAG = 2             # kt8 distance between scores and AV consumption
PSS = 3                # score psum ring


class Filler:
    """Wraps a generator that emits one filler matmul per next()."""

    def __init__(self, gen):
        self.gen = gen
        self.done = False

    def take(self, n):
        for _ in range(n):
            if self.done:
                return
            try:
                next(self.gen)
            except StopIteration:
                self.done = True
                return

    def drain(self):
        while not self.done:
            self.take(1)


def build_nc():
    import concourse.bass as bass
    import concourse.mybir as mybir
    import concourse.tile as tile
    from concourse import bacc

    from concourse import bass_isa

    f32 = mybir.dt.float32
    bf16 = mybir.dt.bfloat16

    nc = bacc.Bacc(None, target_bir_lowering=False, debug=False)

    xT = nc.dram_tensor("xT", [128, KT * S], bf16, kind="ExternalInput")
    wq = nc.dram_tensor("wq", [128, DT * H], bf16, kind="ExternalInput")
    wk = nc.dram_tensor("wk", [128, DT * H], bf16, kind="ExternalInput")
    wv = nc.dram_tensor("wv", [128, 4 * KT * 512], bf16, kind="ExternalInput")
    wo = nc.dram_tensor("wo", [128, DT * H], bf16, kind="ExternalInput")
    cosT = nc.dram_tensor("cosT", [128, S], bf16, kind="ExternalInput")
    sinTs = nc.dram_tensor("sinTs", [128, S], bf16, kind="ExternalInput")
    out = nc.dram_tensor("out", [H, S], bf16, kind="ExternalOutput")

    with tile.TileContext(nc) as tc:
        import contextlib
        with contextlib.ExitStack() as ctx:
            # ---- persistent SBUF tiles -------------------------------------
            persist = ctx.enter_context(tc.tile_pool(name="persist", bufs=1))
            qT_sb = persist.tile([128, NH * S], bf16, tag="qT")
            kT_sb = persist.tile([128, NH * S], bf16, tag="kT")
            v_sb = persist.tile([128, ST * H], bf16, tag="v")
            attnT_sb = persist.tile([128, NH * S], bf16, tag="attnT")
            ones_mat = persist.tile([128, 128], bf16, tag="ones_mat")
            nc.vector.memset(ones_mat[:], 1.0)

            # xT lives through phase A and BV0 (V projection reads it)
            x_pool = ctx.enter_context(tc.tile_pool(name="xpool", bufs=1))
            xT_sb = x_pool.tile([128, KT * S], bf16, tag="xT")

            # wv stream lives A..BV0: ring of half-group chunks (8KB each)
            wv_pool = ctx.enter_context(tc.tile_pool(name="wv_stream", bufs=3))

            def load_wv_chunk(g, half):
                t = wv_pool.tile([128, 8 * 512], bf16, tag="wv",
                                 name=f"wv_{g}_{half}")
                nc.sync.dma_start(
                    out=t[:],
                    in_=wv[:, g * KT * 512 + half * 8 * 512:
                           g * KT * 512 + (half + 1) * 8 * 512])
                return t

            # ---- phase A: Q^T / K^T projection with fused RoPE -------------
            with tc.tile_pool(name="trig", bufs=1) as trig_pool, \
                 tc.tile_pool(name="wqk_stream", bufs=3) as wqk_pool, \
                 tc.tile_pool(name="rope_scratch", bufs=1) as rope_pool, \
                 tc.tile_pool(name="psumA", bufs=6, space="PSUM") as psA:

                cos_sb = trig_pool.tile([128, S], bf16, tag="cos")
                sin_sb = trig_pool.tile([128, S], bf16, tag="sin")

                # startup-critical DMA order: the first kt-halves of the
                # first two q weight tiles and the first x k-tile land
                # first; cos/sin (needed only at first eviction) go last.
                wt_pre = {}
                wt0 = wqk_pool.tile([128, KT * 128], bf16, tag="wqk",
                                    name="wt_pre_0")
                wt1 = wqk_pool.tile([128, KT * 128], bf16, tag="wqk",
                                    name="wt_pre_1")
                nc.sync.dma_start(out=wt0[:, 0:512], in_=wq[:, 0:512])
                nc.sync.dma_start(out=xT_sb[:, 0:S], in_=xT[:, 0:S])
                nc.sync.dma_start(out=wt1[:, 0:512], in_=wq[:, H:H + 512])
                nc.sync.dma_start(out=xT_sb[:, S:2 * S], in_=xT[:, S:2 * S])
                nc.sync.dma_start(out=wt0[:, 512:1024], in_=wq[:, 512:1024])
                nc.sync.dma_start(out=wt1[:, 512:1024],
                                  in_=wq[:, H + 512:H + 1024])
                nc.sync.dma_start(out=wt0[:, 1024:2048], in_=wq[:, 1024:2048])
                nc.sync.dma_start(out=wt1[:, 1024:2048],
                                  in_=wq[:, H + 1024:H + 2048])
                for kt in range(2, KT):
                    nc.sync.dma_start(out=xT_sb[:, kt * S:(kt + 1) * S],
                                      in_=xT[:, kt * S:(kt + 1) * S])
                nc.sync.dma_start(out=cos_sb[:], in_=cosT[:])
                nc.sync.dma_start(out=sin_sb[:], in_=sinTs[:])
                wt_pre[0] = wt0
                wt_pre[1] = wt1
                # prefetch V group 0 so the BV0 prologue starts immediately
                wv_g0 = [load_wv_chunk(0, 0), load_wv_chunk(0, 1)]

                def rope_evict(psum, dst_ap, sc):
                    # dst = psum*cos + shifted(psum)*sin_signed over this chunk
                    cs = cos_sb[:, sc * SCHUNK:(sc + 1) * SCHUNK]
                    ss = sin_sb[:, sc * SCHUNK:(sc + 1) * SCHUNK]
                    m1 = rope_pool.tile([128, SCHUNK], f32, tag="rope_m1")
                    nc.vector.tensor_mul(m1[:], psum[:], cs)
                    m2 = rope_pool.tile([128, SCHUNK], f32, tag="rope_tmp")
                    nc.vector.tensor_mul(m2[0:64, :], psum[64:128, :],
                                         ss[0:64, :])
                    nc.vector.tensor_mul(m2[64:128, :], psum[0:64, :],
                                         ss[64:128, :])
                    nc.vector.tensor_add(dst_ap, m1[:], m2[:])

                # kt-outer warmup over the first two q d-tiles: 4 psum groups
                # accumulate in parallel so each matmul is gated only on its
                # own xT k-tile DMA, not on the whole xT stream.
                warm = []
                for dt in (0, 1):
                    for sc in range(NSC):
                        ps = psA.tile([128, SCHUNK], f32, tag="psA",
                                      name=f"psA_warm_{dt}_{sc}")
                        warm.append((dt, sc, ps))
                for kt in range(KT):
                    for dt, sc, ps in warm:
                        nc.tensor.matmul(
                            ps[:],
                            wt_pre[dt][:, kt * 128:(kt + 1) * 128],
                            xT_sb[:, kt * S + sc * SCHUNK:
                                  kt * S + (sc + 1) * SCHUNK],
                            start=(kt == 0), stop=(kt == KT - 1),
                        )
                for dt, sc, ps in warm:
                    rope_evict(ps, qT_sb[:, dt * S + sc * SCHUNK:
                                         dt * S + (sc + 1) * SCHUNK], sc)

                for which, wdram, dst_sb in (("q", wq, qT_sb), ("k", wk, kT_sb)):
                    for dt in range(DT):
                        if which == "q" and dt in wt_pre:
                            continue  # handled by the kt-outer warmup
                        else:
                            wt = wqk_pool.tile([128, KT * 128], bf16, tag="wqk",
                                               name=f"wt_{which}_{dt}")
                            for c in range(2):
                                nc.sync.dma_start(
                                    out=wt[:, c * 1024:(c + 1) * 1024],
                                    in_=wdram[:, dt * H + c * 1024:
                                              dt * H + (c + 1) * 1024])
                        for sc in range(NSC):
                            ps = psA.tile([128, SCHUNK], f32, tag="psA")
                            for kt in range(KT):
                                nc.tensor.matmul(
                                    ps[:],
                                    wt[:, kt * 128:(kt + 1) * 128],
                                    xT_sb[:, kt * S + sc * SCHUNK:
                                          kt * S + (sc + 1) * SCHUNK],
                                    start=(kt == 0), stop=(kt == KT - 1),
                                )
                            dst = dst_sb[:, dt * S + sc * SCHUNK:
                                         dt * S + (sc + 1) * SCHUNK]
                            rope_evict(ps, dst, sc)

            # ---- attention pools (BV0 + BV1) --------------------------------
            wo_pool = ctx.enter_context(tc.tile_pool(name="wo_stream", bufs=3))

            def load_wo(qc, ot):
                t = wo_pool.tile([128, KT * 128], bf16, tag="wo",
                                 name=f"wo_{qc}_{ot}")
                nc.sync.dma_start(out=t[:], in_=wo[:, ot * H:(ot + 1) * H])
                return t

            with tc.tile_pool(name="expS", bufs=4) as expS_pool, \
                 tc.tile_pool(name="esum", bufs=2) as esum_pool, \
                 tc.tile_pool(name="norm", bufs=1) as norm_pool, \
                 tc.tile_pool(name="psS", bufs=PSS, space="PSUM") as psS, \
                 tc.tile_pool(name="psAV", bufs=2, space="PSUM") as psAV, \
                 tc.tile_pool(name="psDen", bufs=1, space="PSUM") as psDen, \
                 tc.tile_pool(name="psFill", bufs=2, space="PSUM") as psFill:

                def attention(h, qc, filler, f1, f2, ftail, fden):
                    ps_av = psAV.tile([128, SCHUNK], f32, tag="psAV",
                                      name=f"psav_{h}_{qc}")
                    esum = esum_pool.tile([128, SCHUNK], bf16, tag="esum",
                                          name=f"esum_{h}_{qc}")
                    exp_tiles = {}

                    def av(j):
                        nc.tensor.matmul(
                            ps_av[:],
                            v_sb[:, j * H + h * 128: j * H + (h + 1) * 128],
                            exp_tiles[j][:],
                            start=(j == 0), stop=(j == ST - 1),
                        )

                    for kt8 in range(ST):
                        ps_s = psS.tile([128, SCHUNK], f32, tag="psS",
                                        name=f"pss_{h}_{qc}_{kt8}")
                        nc.tensor.matmul(
                            ps_s[:],
                            kT_sb[:, h * S + kt8 * 128: h * S + (kt8 + 1) * 128],
                            qT_sb[:, h * S + qc * SCHUNK:
                                  h * S + (qc + 1) * SCHUNK],
                            start=True, stop=True,
                        )
                        e = expS_pool.tile([128, SCHUNK], bf16, tag="expS",
                                           name=f"exp_{h}_{qc}_{kt8}")
                        nc.scalar.activation(
                            e[:], ps_s[:],
                            func=mybir.ActivationFunctionType.Exp,
                            scale=SCALE,
                        )
                        exp_tiles[kt8] = e
                        if kt8 == 0:
                            nc.vector.tensor_copy(esum[:], e[:])
                        else:
                            nc.vector.tensor_add(esum[:], esum[:], e[:])
                        filler.take(f1)
                        if kt8 >= AV_LAG:
                            av(kt8 - AV_LAG)
                        filler.take(f2)
                    for j in range(ST - AV_LAG, ST):
                        av(j)
                        filler.take(ftail)
                    filler.take(fden)
                    ps_den = psDen.tile([128, SCHUNK], f32, tag="psDen",
                                        name=f"psden_{h}_{qc}")
                    nc.tensor.matmul(ps_den[:], ones_mat[:], esum[:],
                                     start=True, stop=True)
                    recipb = norm_pool.tile([128, SCHUNK], f32, tag="recipb",
                                            name=f"recipb_{h}_{qc}")
                    nc.vector.reciprocal_approx_fast(out=recipb[:], in_=ps_den[:])
                    nc.vector.tensor_mul(
                        attnT_sb[:, h * S + qc * SCHUNK:
                                 h * S + (qc + 1) * SCHUNK],
                        ps_av[:], recipb[:])

                # ---- BV0: attention qc=0, V projection as filler ---------
                def v_gen():
                    chunks = wv_g0
                    for g in range(4):
                        nxt = None
                        for st in range(ST):
                            if st == 4 and g < 3:
                                nxt = [load_wv_chunk(g + 1, 0),
                                       load_wv_chunk(g + 1, 1)]
                            ps = psFill.tile([128, SCHUNK], f32, tag="psF",
                                             name=f"psv_{g}_{st}")
                            for kt in range(KT):
                                ch = chunks[kt // 8]
                                nc.tensor.matmul(
                                    ps[:],
                                    xT_sb[:, kt * S + st * 128:
                                          kt * S + (st + 1) * 128],
                                    ch[:, (kt % 8) * 512:(kt % 8 + 1) * 512],
                                    start=(kt == 0), stop=(kt == KT - 1),
                                )
                                yield
                            # DVE copy: keeps ScalarE free for the exp stream
                            nc.vector.tensor_copy(
                                v_sb[:, st * H + g * 512: st * H + (g + 1) * 512],
                                ps[:])
                        chunks = nxt

                o_pre = {}
                vf = Filler(v_gen())
                vf.take(128)  # V group 0 prologue (heads 0-3 need it)
                for h in range(NH):
                    if h == NH - 2:
                        # prefetch the first two O-proj weight tiles so BV1's
                        # filler doesn't head-of-line-block the PE
                        o_pre[0] = {0: load_wo(0, 0), 1: load_wo(0, 1)}
                    attention(h, 0, vf, f1=2, f2=1, ftail=2, fden=4)
                vf.drain()

                # ---- BV1: attention qc=1, O projection (qc=0) as filler --
                ostage_cm = tc.tile_pool(name="ostage", bufs=2)
                ostage_pool = ostage_cm.__enter__()

                def o_gen(qc, tiles):
                    for ot in range(DT):
                        if ot not in tiles:
                            tiles[ot] = load_wo(qc, ot)
                        wot = tiles.pop(ot)
                        ps = psFill.tile([128, SCHUNK], f32, tag="psF",
                                         name=f"pso_{qc}_{ot}")
                        for dt in range(DT):
                            nc.tensor.matmul(
                                ps[:],
                                wot[:, dt * 128:(dt + 1) * 128],
                                attnT_sb[:, dt * S + qc * SCHUNK:
                                         dt * S + (qc + 1) * SCHUNK],
                                start=(dt == 0), stop=(dt == DT - 1),
                            )
                            if dt == 7 and ot + 2 < DT:
                                tiles[ot + 2] = load_wo(qc, ot + 2)
                            yield
                        o_bf = ostage_pool.tile([128, SCHUNK], bf16,
                                                tag="ostage",
                                                name=f"ost_{qc}_{ot}")
                        nc.scalar.copy(o_bf[:], ps[:])
                        # post from ScalarE right after its own evict: no
                        # cross-engine semaphore, no GpSimd drain at exit
                        nc.scalar.dma_start(
                            out=out[ot * 128:(ot + 1) * 128,
                                    qc * SCHUNK:(qc + 1) * SCHUNK],
                            in_=o_bf[:])

                of0 = Filler(o_gen(0, o_pre[0]))
                for h in range(NH):
                    if h == NH - 2:
                        o_pre[1] = {0: load_wo(1, 0), 1: load_wo(1, 1)}
                    attention(h, 1, of0, f1=1, f2=1, ftail=0, fden=0)
                of0.drain()

                # ---- epilogue: O projection qc=1 --------------------------
                of1 = Filler(o_gen(1, o_pre[1]))
                of1.drain()

                ostage_cm.__exit__(None, None, None)
    nc.finalize()
    return nc


def _prep_core_inputs(x_bc, wq_t, wk_t, wv_t, wo_t, cosT_p, sinTs_p):
    # x_bc: (S, H) f32 -> xT partition-major [128, KT*S] bf16
    xT_p = np.ascontiguousarray(
        x_bc.T.reshape(KT, 128, S).transpose(1, 0, 2).reshape(128, KT * S)
    ).astype(BF)
    return {
        "xT": xT_p, "wq": wq_t, "wk": wk_t, "wv": wv_t, "wo": wo_t,
        "cosT": cosT_p, "sinTs": sinTs_p,
    }


def _prep_shared(cos, sin, w_qkv, w_o):
    def dtile_major(w):  # (H, 2048) -> [128, DT*H], lhsT tiles (dt, kt)
        return np.ascontiguousarray(
            w.reshape(KT, 128, DT, 128).transpose(1, 2, 0, 3).reshape(128, DT * H)
        ).astype(BF)

    wq_t = dtile_major(w_qkv[:, :H])
    wk_t = dtile_major(w_qkv[:, H:2 * H])
    wo_t = dtile_major(w_o)
    wv_t = np.ascontiguousarray(
        w_qkv[:, 2 * H:].reshape(KT, 128, 4, 512).transpose(1, 2, 0, 3)
        .reshape(128, 4 * KT * 512)
    ).astype(BF)

    cos_p = np.ones((S, HD), np.float32)
    cos_p[:NENC] = cos
    sin_p = np.zeros((S, HD), np.float32)
    sin_p[:NENC] = sin
    cosT_p = np.ascontiguousarray(cos_p.T).astype(BF)
    sinT = sin_p.T.copy()
    sinTs_p = np.concatenate([-sinT[:64], sinT[64:]], axis=0)
    sinTs_p = np.ascontiguousarray(sinTs_p).astype(BF)
    return wq_t, wk_t, wv_t, wo_t, cosT_p, sinTs_p


_CACHED_NC = None


def kernel(hidden_states, cos, sin, w_qkv, w_o):
    global _CACHED_NC
    from concourse.bass_utils import run_bass_kernel_spmd

    hidden_states = np.asarray(hidden_states, dtype=np.float32)
    cos = np.asarray(cos, dtype=np.float32)
    sin = np.asarray(sin, dtype=np.float32)
    w_qkv = np.asarray(w_qkv, dtype=np.float32)
    w_o = np.asarray(w_o, dtype=np.float32)

    shared = _prep_shared(cos, sin, w_qkv, w_o)
    xs = hidden_states.reshape(B * C, S, H)
    in_maps = [_prep_core_inputs(xs[i], *shared) for i in range(NCORES)]

    if _CACHED_NC is None:
        _CACHED_NC = build_nc()
    res = run_bass_kernel_spmd(_CACHED_NC, in_maps, list(range(NCORES)))

    out_full = np.empty((B * C, S, H), np.float32)
    for i in range(NCORES):
        out_full[i] = np.asarray(res.results[i]["out"]).astype(np.float32).T
    return out_full.reshape(B, C, S, H)


# revision 39
# speedup vs baseline: 1.3031x; 1.0007x over previous
"""Trainium2 Bass kernel: multi-head attention (B,C,S,H)=(2,4,1024,2048), NH=16, HD=128.

Strategy: pure data-parallel over the 8 B*C batch elements -> 8 NeuronCores,
no collectives.  Per core (v2, software-pipelined phases):
  phase A:  Q^T,K^T projection (transposed [head-dim, seq] layout, lhsT = w
            tiles, rhs = x^T tiles) with RoPE fused into the PSUM eviction.
            Startup DMAs ordered so the first matmul's inputs land first.
  phase BV0: attention for q-chunk 0 of every head, with the V projection
            matmuls interleaved as PE filler so the ScalarE exp stream hides
            behind matmuls instead of stalling the PE.
  phase BV1: attention for q-chunk 1, with the O-projection matmuls for
            q-chunk 0 interleaved as filler (one output d-tile per head).
  epilogue: O-projection for q-chunk 1, outputs DMA'd per-tile in bf16
            (host upcasts to f32).
All matmuls bf16 (fp8 fails the 2e-2 gate: measured 3-6% rel err), f32
accumulation in PSUM.  Host side pre-transposes/pre-tiles/casts inputs so
every DMA is partition-major contiguous.
"""

import numpy as np
import ml_dtypes

try:
    import concourse  # noqa: F401
except ImportError:
    import sys
    sys.path.insert(0, "/opt/trn_rl_repo")

BF = ml_dtypes.bfloat16

B, C, S, H = 2, 4, 1024, 2048
NH, HD, NENC = 16, 128, 1008
NCORES = 8
KT = H // 128          # 16 contraction tiles for the projections
DT = H // 128          # 16 output d-tiles (heads) for Q/K
ST = S // 128          # 8 seq tiles
SCHUNK = 512
NSC = S // SCHUNK      # 2 seq chunks
SCALE = 1.0 / float(np.sqrt(HD))
AV_LAG = 2             # kt8 distance between scores and AV consumption
PSS = 3                # score psum ring


class Filler:
    """Wraps a generator that emits one filler matmul per next()."""

    def __init__(self, gen):
        self.gen = gen
        self.done = False

    def take(self, n):
        for _ in range(n):
            if self.done:
                return
            try:
                next(self.gen)
            except StopIteration:
                self.done = True
                return

    def drain(self):
        while not self.done:
            self.take(1)


def build_nc():
    import concourse.bass as bass
    import concourse.mybir as mybir
    import concourse.tile as tile
    from concourse import bacc

    from concourse import bass_isa

    f32 = mybir.dt.float32
    bf16 = mybir.dt.bfloat16

    nc = bacc.Bacc(None, target_bir_lowering=False, debug=False)

    xT = nc.dram_tensor("xT", [128, KT * S], bf16, kind="ExternalInput")
    wq = nc.dram_tensor("wq", [128, DT * H], bf16, kind="ExternalInput")
    wk = nc.dram_tensor("wk", [128, DT * H], bf16, kind="ExternalInput")
    wv = nc.dram_tensor("wv", [128, 4 * KT * 512], bf16, kind="ExternalInput")
    wo = nc.dram_tensor("wo", [128, DT * H], bf16, kind="ExternalInput")
    cosT = nc.dram_tensor("cosT", [128, S], bf16, kind="ExternalInput")
    sinTs = nc.dram_tensor("sinTs", [128, S], bf16, kind="ExternalInput")
    out = nc.dram_tensor("out", [H, S], bf16, kind="ExternalOutput")

    with tile.TileContext(nc) as tc:
        import contextlib
        with contextlib.ExitStack() as ctx:
            # ---- persistent SBUF tiles -------------------------------------
            persist = ctx.enter_context(tc.tile_pool(name="persist", bufs=1))
            qT_sb = persist.tile([128, NH * S], bf16, tag="qT")
            kT_sb = persist.tile([128, NH * S], bf16, tag="kT")
            v_sb = persist.tile([128, ST * H], bf16, tag="v")
            attnT_sb = persist.tile([128, NH * S], bf16, tag="attnT")
            ones_mat = persist.tile([128, 128], bf16, tag="ones_mat")
            nc.vector.memset(ones_mat[:], 1.0)

            # xT lives through phase A and BV0 (V projection reads it)
            x_pool = ctx.enter_context(tc.tile_pool(name="xpool", bufs=1))
            xT_sb = x_pool.tile([128, KT * S], bf16, tag="xT")

            # wv stream lives A..BV0: ring of half-group chunks (8KB each)
            wv_pool = ctx.enter_context(tc.tile_pool(name="wv_stream", bufs=3))

            def load_wv_chunk(g, half):
                t = wv_pool.tile([128, 8 * 512], bf16, tag="wv",
                                 name=f"wv_{g}_{half}")
                nc.sync.dma_start(
                    out=t[:],
                    in_=wv[:, g * KT * 512 + half * 8 * 512:
                           g * KT * 512 + (half + 1) * 8 * 512])
                return t

            # ---- phase A: Q^T / K^T projection with fused RoPE -------------
            with tc.tile_pool(name="trig", bufs=1) as trig_pool, \
                 tc.tile_pool(name="wqk_stream", bufs=3) as wqk_pool, \
                 tc.tile_pool(name="rope_scratch", bufs=1) as rope_pool, \
                 tc.tile_pool(name="psumA", bufs=6, space="PSUM") as psA:

                cos_sb = trig_pool.tile([128, S], bf16, tag="cos")
                sin_sb = trig_pool.tile([128, S], bf16, tag="sin")

                # startup-critical DMA order: the first kt-halves of the
                # first two q weight tiles and the first x k-tile land
                # first; cos/sin (needed only at first eviction) go last.
                wt_pre = {}
                wt0 = wqk_pool.tile([128, KT * 128], bf16, tag="wqk",
                                    name="wt_pre_0")
                wt1 = wqk_pool.tile([128, KT * 128], bf16, tag="wqk",
                                    name="wt_pre_1")
                nc.sync.dma_start(out=wt0[:, 0:512], in_=wq[:, 0:512])
                nc.sync.dma_start(out=xT_sb[:, 0:S], in_=xT[:, 0:S])
                nc.sync.dma_start(out=wt1[:, 0:512], in_=wq[:, H:H + 512])
                nc.sync.dma_start(out=xT_sb[:, S:2 * S], in_=xT[:, S:2 * S])
                nc.sync.dma_start(out=wt0[:, 512:1024], in_=wq[:, 512:1024])
                nc.sync.dma_start(out=wt1[:, 512:1024],
                                  in_=wq[:, H + 512:H + 1024])
                nc.sync.dma_start(out=wt0[:, 1024:2048], in_=wq[:, 1024:2048])
                nc.sync.dma_start(out=wt1[:, 1024:2048],
                                  in_=wq[:, H + 1024:H + 2048])
                for kt in range(2, KT):
                    nc.sync.dma_start(out=xT_sb[:, kt * S:(kt + 1) * S],
                                      in_=xT[:, kt * S:(kt + 1) * S])
                nc.sync.dma_start(out=cos_sb[:], in_=cosT[:])
                nc.sync.dma_start(out=sin_sb[:], in_=sinTs[:])
                wt_pre[0] = wt0
                wt_pre[1] = wt1
                # prefetch V group 0 so the BV0 prologue starts immediately
                wv_g0 = [load_wv_chunk(0, 0), load_wv_chunk(0, 1)]

                def rope_evict(psum, dst_ap, sc):
                    # dst = psum*cos + shifted(psum)*sin_signed over this chunk
                    cs = cos_sb[:, sc * SCHUNK:(sc + 1) * SCHUNK]
                    ss = sin_sb[:, sc * SCHUNK:(sc + 1) * SCHUNK]
                    m1 = rope_pool.tile([128, SCHUNK], f32, tag="rope_m1")
                    nc.vector.tensor_mul(m1[:], psum[:], cs)
                    m2 = rope_pool.tile([128, SCHUNK], f32, tag="rope_tmp")
                    nc.vector.tensor_mul(m2[0:64, :], psum[64:128, :],
                                         ss[0:64, :])
                    nc.vector.tensor_mul(m2[64:128, :], psum[0:64, :],
                                         ss[64:128, :])
                    nc.vector.tensor_add(dst_ap, m1[:], m2[:])

                # kt-outer warmup over the first two q d-tiles: 4 psum groups
                # accumulate in parallel so each matmul is gated only on its
                # own xT k-tile DMA, not on the whole xT stream.
                warm = []
                for dt in (0, 1):
                    for sc in range(NSC):
                        ps = psA.tile([128, SCHUNK], f32, tag="psA",
                                      name=f"psA_warm_{dt}_{sc}")
                        warm.append((dt, sc, ps))
                for kt in range(KT):
                    for dt, sc, ps in warm:
                        nc.tensor.matmul(
                            ps[:],
                            wt_pre[dt][:, kt * 128:(kt + 1) * 128],
                            xT_sb[:, kt * S + sc * SCHUNK:
                                  kt * S + (sc + 1) * SCHUNK],
                            start=(kt == 0), stop=(kt == KT - 1),
                        )
                for dt, sc, ps in warm:
                    rope_evict(ps, qT_sb[:, dt * S + sc * SCHUNK:
                                         dt * S + (sc + 1) * SCHUNK], sc)

                for which, wdram, dst_sb in (("q", wq, qT_sb), ("k", wk, kT_sb)):
                    for dt in range(DT):
                        if which == "q" and dt in wt_pre:
                            continue  # handled by the kt-outer warmup
                        else:
                            wt = wqk_pool.tile([128, KT * 128], bf16, tag="wqk",
                                               name=f"wt_{which}_{dt}")
                            for c in range(2):
                                nc.sync.dma_start(
                                    out=wt[:, c * 1024:(c + 1) * 1024],
                                    in_=wdram[:, dt * H + c * 1024:
                                              dt * H + (c + 1) * 1024])
                        for sc in range(NSC):
                            ps = psA.tile([128, SCHUNK], f32, tag="psA")
                            for kt in range(KT):
                                nc.tensor.matmul(
                                    ps[:],
                                    wt[:, kt * 128:(kt + 1) * 128],
                                    xT_sb[:, kt * S + sc * SCHUNK:
                                          kt * S + (sc + 1) * SCHUNK],
                                    start=(kt == 0), stop=(kt == KT - 1),
                                )
                            dst = dst_sb[:, dt * S + sc * SCHUNK:
                                         dt * S + (sc + 1) * SCHUNK]
                            rope_evict(ps, dst, sc)

            # ---- attention pools (BV0 + BV1) --------------------------------
            wo_pool = ctx.enter_context(tc.tile_pool(name="wo_stream", bufs=3))

            def load_wo(qc, ot):
                t = wo_pool.tile([128, KT * 128], bf16, tag="wo",
                                 name=f"wo_{qc}_{ot}")
                nc.sync.dma_start(out=t[:], in_=wo[:, ot * H:(ot + 1) * H])
                return t

            with tc.tile_pool(name="expS", bufs=4) as expS_pool, \
                 tc.tile_pool(name="esum", bufs=2) as esum_pool, \
                 tc.tile_pool(name="norm", bufs=1) as norm_pool, \
                 tc.tile_pool(name="psS", bufs=PSS, space="PSUM") as psS, \
                 tc.tile_pool(name="psAV", bufs=2, space="PSUM") as psAV, \
                 tc.tile_pool(name="psDen", bufs=1, space="PSUM") as psDen, \
                 tc.tile_pool(name="psFill", bufs=2, space="PSUM") as psFill:

                def attention(h, qc, filler, f1, f2, ftail, fden):
                    ps_av = psAV.tile([128, SCHUNK], f32, tag="psAV",
                                      name=f"psav_{h}_{qc}")
                    esum = esum_pool.tile([128, SCHUNK], bf16, tag="esum",
                                          name=f"esum_{h}_{qc}")
                    exp_tiles = {}

                    def av(j):
                        nc.tensor.matmul(
                            ps_av[:],
                            v_sb[:, j * H + h * 128: j * H + (h + 1) * 128],
                            exp_tiles[j][:],
                            start=(j == 0), stop=(j == ST - 1),
                        )

                    for kt8 in range(ST):
                        ps_s = psS.tile([128, SCHUNK], f32, tag="psS",
                                        name=f"pss_{h}_{qc}_{kt8}")
                        nc.tensor.matmul(
                            ps_s[:],
                            kT_sb[:, h * S + kt8 * 128: h * S + (kt8 + 1) * 128],
                            qT_sb[:, h * S + qc * SCHUNK:
                                  h * S + (qc + 1) * SCHUNK],
                            start=True, stop=True,
                        )
                        e = expS_pool.tile([128, SCHUNK], bf16, tag="expS",
                                           name=f"exp_{h}_{qc}_{kt8}")
                        nc.scalar.activation(
                            e[:], ps_s[:],
                            func=mybir.ActivationFunctionType.Exp,
                            scale=SCALE,
                        )
                        exp_tiles[kt8] = e
                        if kt8 == 0:
                            nc.vector.tensor_copy(esum[:], e[:])
                        else:
                            nc.vector.tensor_add(esum[:], esum[:], e[:])
                        filler.take(f1)
                        if kt8 >= AV_LAG:
                            av(kt8 - AV_LAG)
                        filler.take(f2)
                    for j in range(ST - AV_LAG, ST):
                        av(j)
                        filler.take(ftail)
                    filler.take(fden)
                    ps_den = psDen.tile([128, SCHUNK], f32, tag="psDen",
                                        name=f"psden_{h}_{qc}")
                    nc.tensor.matmul(ps_den[:], ones_mat[:], esum[:],
                                     start=True, stop=True)
                    recipb = norm_pool.tile([128, SCHUNK], f32, tag="recipb",
                                            name=f"recipb_{h}_{qc}")
                    nc.vector.reciprocal_approx_fast(out=recipb[:], in_=ps_den[:])
                    nc.vector.tensor_mul(
                        attnT_sb[:, h * S + qc * SCHUNK:
                                 h * S + (qc + 1) * SCHUNK],
                        ps_av[:], recipb[:])

                # ---- BV0: attention qc=0, V projection as filler ---------
                def v_gen():
                    chunks = wv_g0
                    for g in range(4):
                        nxt = None
                        for st in range(ST):
                            if st == 4 and g < 3:
                                nxt = [load_wv_chunk(g + 1, 0),
                                       load_wv_chunk(g + 1, 1)]
                            ps = psFill.tile([128, SCHUNK], f32, tag="psF",
                                             name=f"psv_{g}_{st}")
                            for kt in range(KT):
                                ch = chunks[kt // 8]
                                nc.tensor.matmul(
                                    ps[:],
                                    xT_sb[:, kt * S + st * 128:
                                          kt * S + (st + 1) * 128],
                                    ch[:, (kt % 8) * 512:(kt % 8 + 1) * 512],
                                    start=(kt == 0), stop=(kt == KT - 1),
                                )
                                yield
                            # DVE copy: keeps ScalarE free for the exp stream
                            nc.vector.tensor_copy(
                                v_sb[:, st * H + g * 512: st * H + (g + 1) * 512],
                                ps[:])
                        chunks = nxt

                o_pre = {}
                vf = Filler(v_gen())
                vf.take(128)  # V group 0 prologue (heads 0-3 need it)
                for h in range(NH):
                    if h == NH - 2:
                        # prefetch the first two O-proj weight tiles so BV1's
                        # filler doesn't head-of-line-block the PE
                        o_pre[0] = {0: load_wo(0, 0), 1: load_wo(0, 1)}
                    attention(h, 0, vf, f1=2, f2=1, ftail=2, fden=4)
                vf.drain()

                # ---- BV1: attention qc=1, O projection (qc=0) as filler --
                ostage_cm = tc.tile_pool(name="ostage", bufs=2)
                ostage_pool = ostage_cm.__enter__()

                def o_gen(qc, tiles):
                    for ot in range(DT):
                        if ot not in tiles:
                            tiles[ot] = load_wo(qc, ot)
                        wot = tiles.pop(ot)
                        ps = psFill.tile([128, SCHUNK], f32, tag="psF",
                                         name=f"pso_{qc}_{ot}")
                        for dt in range(DT):
                            nc.tensor.matmul(
                                ps[:],
                                wot[:, dt * 128:(dt + 1) * 128],
                                attnT_sb[:, dt * S + qc * SCHUNK:
                                         dt * S + (qc + 1) * SCHUNK],
                                start=(dt == 0), stop=(dt == DT - 1),
                            )
                            if dt == 7 and ot + 2 < DT:
                                tiles[ot + 2] = load_wo(qc, ot + 2)
                            yield
                        o_bf = ostage_pool.tile([128, SCHUNK], bf16,
                                                tag="ostage",
                                                name=f"ost_{qc}_{ot}")
                        nc.scalar.copy(o_bf[:], ps[:])
                        # post from ScalarE right after its own evict: no
                        # cross-engine semaphore, no GpSimd drain at exit
                        nc.scalar.dma_start(
                            out=out[ot * 128:(ot + 1) * 128,
                                    qc * SCHUNK:(qc + 1) * SCHUNK],
                            in_=o_bf[:])

                of0 = Filler(o_gen(0, o_pre[0]))
                for h in range(NH):
                    if h == NH - 2:
                        o_pre[1] = {0: load_wo(1, 0), 1: load_wo(1, 1)}
                    attention(h, 1, of0, f1=1, f2=1, ftail=0, fden=0)
                of0.drain()

                # ---- epilogue: O projection qc=1 --------------------------
                of1 = Filler(o_gen(1, o_pre[1]))
                of1.drain()

                ostage_cm.__exit__(None, None, None)
    nc.finalize()
    return nc


def _prep_core_inputs(x_bc, wq_t, wk_t, wv_t, wo_t, cosT_p, sinTs_p):
    # x_bc: (S, H) f32 -> xT partition-major [128, KT*S] bf16
    xT_p = np.ascontiguousarray(
        x_bc.T.reshape(KT, 128, S).transpose(1, 0, 2).reshape(128, KT * S)
    ).astype(BF)
    return {
        "xT": xT_p, "wq": wq_t, "wk": wk_t, "wv": wv_t, "wo": wo_t,
        "cosT": cosT_p, "sinTs": sinTs_p,
    }


def _prep_shared(cos, sin, w_qkv, w_o):
    def dtile_major(w):  # (H, 2048) -> [128, DT*H], lhsT tiles (dt, kt)
        return np.ascontiguousarray(
            w.reshape(KT, 128, DT, 128).transpose(1, 2, 0, 3).reshape(128, DT * H)
        ).astype(BF)

    wq_t = dtile_major(w_qkv[:, :H])
    wk_t = dtile_major(w_qkv[:, H:2 * H])
    wo_t = dtile_major(w_o)
    wv_t = np.ascontiguousarray(
        w_qkv[:, 2 * H:].reshape(KT, 128, 4, 512).transpose(1, 2, 0, 3)
        .reshape(128, 4 * KT * 512)
    ).astype(BF)

    cos_p = np.ones((S, HD), np.float32)
    cos_p[:NENC] = cos
    sin_p = np.zeros((S, HD), np.float32)
    sin_p[:NENC] = sin
    cosT_p = np.ascontiguousarray(cos_p.T).astype(BF)
    sinT = sin_p.T.copy()
    sinTs_p = np.concatenate([-sinT[:64], sinT[64:]], axis=0)
    sinTs_p = np.ascontiguousarray(sinTs_p).astype(BF)
    return wq_t, wk_t, wv_t, wo_t, cosT_p, sinTs_p


_CACHED_NC = None


def kernel(hidden_states, cos, sin, w_qkv, w_o):
    global _CACHED_NC
    from concourse.bass_utils import run_bass_kernel_spmd

    hidden_states = np.asarray(hidden_states, dtype=np.float32)
    cos = np.asarray(cos, dtype=np.float32)
    sin = np.asarray(sin, dtype=np.float32)
    w_qkv = np.asarray(w_qkv, dtype=np.float32)
    w_o = np.asarray(w_o, dtype=np.float32)

    shared = _prep_shared(cos, sin, w_qkv, w_o)
    xs = hidden_states.reshape(B * C, S, H)
    in_maps = [_prep_core_inputs(xs[i], *shared) for i in range(NCORES)]

    if _CACHED_NC is None:
        _CACHED_NC = build_nc()
    res = run_bass_kernel_spmd(_CACHED_NC, in_maps, list(range(NCORES)))

    out_full = np.empty((B * C, S, H), np.float32)
    for i in range(NCORES):
        out_full[i] = np.asarray(res.results[i]["out"]).astype(np.float32).T
    return out_full.reshape(B, C, S, H)
